# revision 39
# baseline (speedup 1.0000x reference)
"""Multi-head attention on 8 Trainium2 NeuronCores.

Problem: B=2, S=2048, D=1024, H=16 heads (head_dim 64), boolean mask,
per-head gate, QKV/out linear projections.

Sharding: core c handles batch b=c//4 and heads 4*(c%4)..4*(c%4)+3.
Each core computes its 4 heads' attention and the partial output
projection (contribution of its 256 concat columns through Wo); the host
sums the 4 partials per batch and adds the constant terms (bo, and the
bv/gate contribution which is constant because attention rows sum to 1).

Device-side layout choices (see comments inline):
  - scores are computed TRANSPOSED [sk, sq] so that softmax needs no
    free-dim reductions at all: exp is a pure elementwise ACT pass,
    the mask is a multiplicative bf16 tensor_mul, and the softmax
    denominator is obtained for free as a 65th "ones" column of the
    PV matmul's stationary operand.
  - all matmuls run in bf16 (1 cycle/row on the PE).
  - normalization divides the PV accumulator by the denominator row via
    reciprocal_approx_fast + gpsimd partition_broadcast + tensor_mul.
"""

import sys

if "/opt/trn_rl_repo" not in sys.path:
    sys.path.insert(0, "/opt/trn_rl_repo")

import numpy as np
import ml_dtypes

import concourse.bass as bass
import concourse.bacc as bacc
import concourse.mybir as mybir
import concourse.tile as tile
from concourse.bass_utils import run_bass_kernel_spmd

BF16 = mybir.dt.float16  # fp16: same speed as bf16, 3 more mantissa bits
F32 = mybir.dt.float32
NPBF16 = np.float16

P = 128
B, S, D = 2, 2048, 1024
HEADS, HD = 16, 64
NCORES = 8
NH = HEADS // (NCORES // B)  # heads per core = 4
COLS = NH * HD               # 256 concat columns per core
DK = D // P                  # 8 contraction chunks for the projections
SKT = S // P                 # 16 key chunks
SQB = 1024                   # query block width in the attention loop
NSQB = S // SQB

_CACHE = {}


def _build_program():
    nc = bacc.Bacc("TRN2", debug=False)

    xqT = nc.declare_dram_parameter("xqT", [D, S], BF16, isOutput=False)
    xkT = nc.declare_dram_parameter("xkT", [D, S], BF16, isOutput=False)
    xvT = nc.declare_dram_parameter("xvT", [D, S], BF16, isOutput=False)
    mT = nc.declare_dram_parameter("mT", [S, S], BF16, isOutput=False)
    wq = nc.declare_dram_parameter("wq", [D, COLS], BF16, isOutput=False)
    wk = nc.declare_dram_parameter("wk", [D, COLS], BF16, isOutput=False)
    wv = nc.declare_dram_parameter("wv", [D, COLS], BF16, isOutput=False)
    wo = nc.declare_dram_parameter("wo", [COLS, D], BF16, isOutput=False)
    bq = nc.declare_dram_parameter("bq", [COLS, 1], F32, isOutput=False)
    bk = nc.declare_dram_parameter("bk", [COLS, 1], F32, isOutput=False)
    od = nc.declare_dram_parameter("od", [D, S], F32, isOutput=True)

    xqT3 = xqT[:].rearrange("(n p) s -> n p s", p=P)
    xkT3 = xkT[:].rearrange("(n p) s -> n p s", p=P)
    xvT3 = xvT[:].rearrange("(n p) s -> n p s", p=P)
    mT3 = mT[:].rearrange("(n p) s -> n p s", p=P)
    wq3 = wq[:].rearrange("(n p) c -> n p c", p=P)
    wk3 = wk[:].rearrange("(n p) c -> n p c", p=P)
    wv3 = wv[:].rearrange("(n p) c -> n p c", p=P)
    wo3 = wo[:].rearrange("(n p) d -> n p d", p=P)
    bq3 = bq[:].rearrange("(n p) o -> n p o", p=P)
    bk3 = bk[:].rearrange("(n p) o -> n p o", p=P)
    od3 = od[:].rearrange("(n p) s -> n p s", p=P)

    with tile.TileContext(nc) as tc:
        with (
            tc.tile_pool(name="wpool", bufs=1) as wpool,
            tc.tile_pool(name="qkpool", bufs=1) as qkpool,
            tc.tile_pool(name="vpool", bufs=1) as vpool,
            tc.tile_pool(name="maskpool", bufs=1) as maskpool,
            tc.tile_pool(name="cpool", bufs=1) as cpool,
            tc.tile_pool(name="xpool", bufs=1) as xpool,
            tc.tile_pool(name="pmpool", bufs=1) as pmpool,
            tc.tile_pool(name="npool", bufs=1) as npool,
            tc.tile_pool(name="opool", bufs=1) as opool,
        ):
            # ---- resident weights / biases ----
            wq_sb, wk_sb, wv_sb = [], [], []
            for i in range(DK):
                t = wpool.tile([P, COLS], BF16, name=f"wq_sb{i}")
                nc.gpsimd.dma_start(out=t[:], in_=wq3[i])
                wq_sb.append(t)
            for i in range(DK):
                t = wpool.tile([P, COLS], BF16, name=f"wk_sb{i}")
                nc.gpsimd.dma_start(out=t[:], in_=wk3[i])
                wk_sb.append(t)
            for i in range(DK):
                t = wpool.tile([P, COLS], BF16, name=f"wv_sb{i}")
                nc.gpsimd.dma_start(out=t[:], in_=wv3[i])
                wv_sb.append(t)
            wo_sb = []
            for i in range(COLS // P):
                t = wpool.tile([P, D], BF16, name=f"wo_sb{i}")
                nc.gpsimd.dma_start(out=t[:], in_=wo3[i])
                wo_sb.append(t)
            b_sb = {}
            for nm, src in (("bq", bq3), ("bk", bk3)):
                for i in range(COLS // P):
                    t = wpool.tile([P, 1], F32, name=f"{nm}_sb{i}")
                    nc.gpsimd.dma_start(out=t[:], in_=src[i])
                    b_sb[(nm, i)] = t

            # concat^T (normalized attention outputs, head-major columns)
            concat_sb = [
                cpool.tile([P, S], BF16, name=f"concat_sb{i}")
                for i in range(COLS // P)
            ]

            # ---- Q/K projections: qhT[c, s] = (q @ Wq + bq)^T ----
            # lhsT = Wq chunk [128d, 128c] (stationary), rhs = xT chunk
            # [128d, 512s] -> psum [128c, 512s]; accumulate over 8 d-chunks.
            qhT_sb = {}
            with tc.tile_pool(name="ps_proj", bufs=1, space="PSUM") as psp:
                for tname, x3, w_sb, dest in (
                    ("q", xqT3, wq_sb, "qhT"),
                    ("k", xkT3, wk_sb, "khT"),
                ):
                    ps = [
                        psp.tile([P, S], F32, name=f"psp{c}", tag=f"psp{c}")
                        for c in range(COLS // P)
                    ]
                    for dk in range(DK):
                        xt = xpool.tile([P, S], BF16, name="xt", tag="xt", bufs=8)
                        xeng = nc.scalar if tname == "k" else nc.sync
                        xeng.dma_start(out=xt[:], in_=x3[dk])
                        for c in range(COLS // P):
                            for sb in range(S // 512):
                                nc.tensor.matmul(
                                    ps[c][:, sb * 512 : (sb + 1) * 512],
                                    lhsT=w_sb[dk][:, c * P : (c + 1) * P],
                                    rhs=xt[:, sb * 512 : (sb + 1) * 512],
                                    start=(dk == 0),
                                    stop=(dk == DK - 1),
                                )
                    for c in range(COLS // P):
                        t = qkpool.tile([P, S], BF16, name=f"{tname}hT{c}")
                        nc.vector.tensor_scalar_add(
                            t[:], ps[c][:], b_sb[(f"b{tname}", c)][:]
                        )
                        qhT_sb[(tname, c)] = t

            # mask tiles, resident for the whole attention phase
            # (emitted after Q/K so the scalar queue serves xk first)
            m_sb = []
            for i in range(SKT):
                t = maskpool.tile([P, S], BF16, name=f"m_sb{i}")
                nc.scalar.dma_start(out=t[:], in_=mT3[i])
                m_sb.append(t)

            # ---- V projection: vh[s, c] natural layout, + ones column ----
            # lhsT = xvT chunk [128d, 128s] (stationary), rhs = Wv chunk
            # [128d, 256c] -> psum [128s, 256c]; accumulate over d-chunks.
            # dk-outer so each xv tile is DMA'd once; the 16 per-skt psum
            # accumulators are half a bank each (16 x [128,256]f32 = 8 banks).
            # Two half-phases of 8 skt tiles each (one PSUM bank per skt);
            # xv tiles are streamed again for the second half (cheap DMA).
            vh_sb = [None] * SKT
            with tc.tile_pool(name="ps_v", bufs=1, space="PSUM") as psv_pool:
                for half in range(2):
                    skts = range(half * SKT // 2, (half + 1) * SKT // 2)
                    psv = {
                        skt: psv_pool.tile(
                            [P, COLS], F32, name=f"psv{skt % 8}", tag=f"psv{skt % 8}"
                        )
                        for skt in skts
                    }
                    for dk in range(DK):
                        xt = xpool.tile([P, S], BF16, name="xt", tag="xt", bufs=8)
                        nc.sync.dma_start(out=xt[:], in_=xvT3[dk])
                        for skt in skts:
                            nc.tensor.matmul(
                                psv[skt][:],
                                lhsT=xt[:, skt * P : (skt + 1) * P],
                                rhs=wv_sb[dk][:],
                                start=(dk == 0),
                                stop=(dk == DK - 1),
                            )
                    for skt in skts:
                        vt = vpool.tile([P, NH, HD + 1], BF16, name=f"vh_sb{skt}")
                        nc.vector.tensor_copy(
                            vt[:, :, 0:HD],
                            psv[skt][:].rearrange("p (h d) -> p h d", h=NH),
                        )
                        nc.vector.memset(vt[:, :, HD], 1.0)
                        vh_sb[skt] = vt

            # ---- attention + interleaved output projection ----
            # sqb-outer / head-inner; after each sq block's 4 heads finish,
            # its slice of the output projection runs on psum tiles that
            # share the PV pool tag, so the O-proj of block i overlaps the
            # attention of block i+1 instead of forming a serial tail.
            with (
                tc.tile_pool(name="ps_s", bufs=1, space="PSUM") as ps_s_pool,
                tc.tile_pool(name="ps_pv", bufs=1, space="PSUM") as ps_pv_pool,
            ):
                def emit_oproj(sqb, dcs):
                    q0 = sqb * SQB
                    for dc in dcs:
                        if sqb == NSQB - 1 and dc % 2 == 1:
                            po = ps_s_pool.tile([P, SQB], F32, name="pso2", tag="pss", bufs=2)
                        else:
                            po = ps_pv_pool.tile([P, SQB], F32, name="pso", tag="pso", bufs=1)
                        for sb in range(SQB // 512):
                            for cc in range(COLS // P):
                                nc.tensor.matmul(
                                    po[:, sb * 512 : (sb + 1) * 512],
                                    lhsT=wo_sb[cc][:, dc * P : (dc + 1) * P],
                                    rhs=concat_sb[cc][:, q0 + sb * 512 : q0 + (sb + 1) * 512],
                                    start=(cc == 0),
                                    stop=(cc == COLS // P - 1),
                                )
                        oev = opool.tile([P, SQB], F32, name="oev", tag="oev", bufs=3)
                        if sqb == NSQB - 1 or dc % 2 == 1:
                            nc.scalar.copy(oev[:], po[:])
                        else:
                            nc.vector.tensor_copy(oev[:], po[:])
                        nc.sync.dma_start(out=od3[dc][:, q0 : q0 + SQB], in_=oev[:])

                for sqb in range(NSQB):
                    q0 = sqb * SQB
                    for h in range(NH):
                        ht, hp = h // 2, HD * (h % 2)
                        qT = qhT_sb[("q", ht)]
                        kT = qhT_sb[("k", ht)]
                        pv = ps_pv_pool.tile(
                            [HD + 1, SQB], F32, name="pspv", tag="pspv", bufs=1
                        )
                        for skc in range(SKT):
                            ss = ps_s_pool.tile(
                                [P, SQB], F32, name="pss", tag="pss", bufs=2
                            )
                            for i in range(SQB // 512):
                                nc.tensor.matmul(
                                    ss[:, i * 512 : (i + 1) * 512],
                                    lhsT=kT[hp : hp + HD, skc * P : (skc + 1) * P],
                                    rhs=qT[hp : hp + HD, q0 + i * 512 : q0 + (i + 1) * 512],
                                    start=True,
                                    stop=True,
                                )
                            pm = pmpool.tile([P, SQB], BF16, name="pm", tag="pm", bufs=3)
                            nc.scalar.activation(
                                pm[:], ss[:], mybir.ActivationFunctionType.Exp
                            )
                            nc.vector.tensor_mul(
                                pm[:], pm[:], m_sb[skc][:, q0 : q0 + SQB]
                            )
                            for i in range(SQB // 512):
                                nc.tensor.matmul(
                                    pv[:, i * 512 : (i + 1) * 512],
                                    lhsT=vh_sb[skc][:, h, :],
                                    rhs=pm[:, i * 512 : (i + 1) * 512],
                                    start=(skc == 0),
                                    stop=(skc == SKT - 1),
                                )
                        # Evacuate the whole PV accumulator to SBUF in one
                        # FD-bound copy (same cost as copying just the denom
                        # row), freeing the psum slot immediately; the rest of
                        # the normalization runs off the SBUF copy.
                        # reciprocal_approx_fast / partition_broadcast only
                        # work on HW for partition-0-based APs, so stage the
                        # denominator row down to partition 0 via a DMA hop.
                        dnc = npool.tile([HD + 1, SQB], F32, name="dnc", tag="dnc", bufs=2)
                        nc.vector.tensor_copy(dnc[:], pv[:])
                        dn0 = npool.tile([1, SQB], F32, name="dn0", tag="dn0", bufs=2)
                        nc.gpsimd.dma_start(out=dn0[:], in_=dnc[HD : HD + 1, :])
                        dnr = npool.tile([1, SQB], F32, name="dnr", tag="dnr", bufs=2)
                        nc.vector.reciprocal_approx_fast(out=dnr[:], in_=dn0[:])
                        rb = npool.tile([HD, SQB], F32, name="rb", tag="rb", bufs=2)
                        nc.gpsimd.partition_broadcast(rb[:], dnr[:])
                        if h % 2 == 0:
                            nc.vector.tensor_mul(
                                concat_sb[ht][0:HD, q0 : q0 + SQB], dnc[0:HD, :], rb[:]
                            )
                        else:
                            tmp = npool.tile([HD, SQB], BF16, name="tmpn", tag="tmpn", bufs=2)
                            nc.vector.tensor_mul(tmp[:], dnc[0:HD, :], rb[:])
                            nc.gpsimd.dma_start(
                                out=concat_sb[ht][HD:P, q0 : q0 + SQB], in_=tmp[:]
                            )
                    # output projection for this sq block (overlaps the
                    # next block's attention via the shared psum slots)
                    emit_oproj(sqb, range(D // P))

    nc.compile()
    return nc


def get_program():
    if "nc" not in _CACHE:
        _CACHE["nc"] = _build_program()
    return _CACHE["nc"]


def make_in_maps(q, k, v, mask, Wq, bq, Wk, bk, Wv, bv, Wo, bo, gate):
    """Host-side sharding: per-core input dict (all numpy)."""
    q, k, v = (np.asarray(a, np.float32) for a in (q, k, v))
    mask = np.asarray(mask)
    Wq, bq, Wk, bk, Wv, bv, Wo, bo, gate = (
        np.asarray(a, np.float32) for a in (Wq, bq, Wk, bk, Wv, bv, Wo, bo, gate)
    )
    scale = 1.0 / np.sqrt(HD)
    xT = {}
    for b in range(B):
        xT[("q", b)] = np.ascontiguousarray(q[b].T).astype(NPBF16)
        xT[("k", b)] = np.ascontiguousarray(k[b].T).astype(NPBF16)
        xT[("v", b)] = np.ascontiguousarray(v[b].T).astype(NPBF16)
        xT[("m", b)] = np.ascontiguousarray(mask[b].T).astype(NPBF16)

    in_maps = []
    for c in range(NCORES):
        b = c // (NCORES // B)
        g = c % (NCORES // B)
        cols = slice(g * COLS, (g + 1) * COLS)
        gate_cols = np.repeat(gate[g * NH : (g + 1) * NH], HD)  # [256]
        in_maps.append(
            {
                "xqT": xT[("q", b)],
                "xkT": xT[("k", b)],
                "xvT": xT[("v", b)],
                "mT": xT[("m", b)],
                # fold the 1/sqrt(hd) score scale into Wq and bq;
                # fold the per-head gate into Wv (bv handled on host)
                "wq": (Wq[:, cols] * scale).astype(NPBF16),
                "wk": Wk[:, cols].astype(NPBF16),
                "wv": (Wv[:, cols] * gate_cols[None, :]).astype(NPBF16),
                "wo": np.ascontiguousarray(Wo[cols, :]).astype(NPBF16),
                "bq": (bq[cols] * scale).astype(np.float32).reshape(COLS, 1),
                "bk": bk[cols].astype(np.float32).reshape(COLS, 1),
            }
        )
    return in_maps


LAST_RESULTS = None


def kernel(q, k, v, mask, Wq, bq, Wk, bk, Wv, bv, Wo, bo, gate, trace=False):
    global LAST_RESULTS
    nc = get_program()
    in_maps = make_in_maps(q, k, v, mask, Wq, bq, Wk, bk, Wv, bv, Wo, bo, gate)
    res = run_bass_kernel_spmd(nc, in_maps, core_ids=list(range(NCORES)), trace=trace)
    LAST_RESULTS = res

    bv_ = np.asarray(bv, np.float32)
    bo_ = np.asarray(bo, np.float32)
    gate_ = np.asarray(gate, np.float32)
    Wo_ = np.asarray(Wo, np.float32)
    # attention rows sum to 1, so the bv term is a constant vector:
    # concat-level constant = repeat(gate, hd) * bv, projected through Wo.
    const = (np.repeat(gate_, HD) * bv_) @ Wo_ + bo_

    out = np.zeros((B, S, D), np.float32)
    for c in range(NCORES):
        b = c // (NCORES // B)
        out[b] += res.results[c]["od"].T
    out += const[None, None, :]
    return out
